# revision 19
# baseline (speedup 1.0000x reference)
"""Trainium2 Bass kernel for the BVPModel Helmholtz-residual PINN problem.

Computes, for N=131072 collocation points, the residual of a Helmholtz-type
PDE through a 4-256-256-256-2 tanh MLP, including the beta-weighted sum of
diagonal second derivatives w.r.t. the three spatial inputs.

Math: forward-mode second-order jets (forward Laplacian). Per point:
  - primal       h_l
  - tangents     u_l^d (scaled by sqrt(beta_d), d=x,y,z)
  - second-order v_l   (beta-weighted; sign/scale folded into head)
through the layers:
  affine:  hat = W h,   uhat_d = W u_d,   vhat = W v
  tanh:    z = tanh(hat+b), s = 1 - z^2
           u_d = s * uhat_d
           v   = (z * sum_d uhat_d^2 + vhat) * s
Folds (all host-side, free on device):
  - a1 = W1 @ [x,y,z,f] + b1 computed on host, streamed in as fp16
  - uhat2_d = (W2 diag(chat_d)) @ s1  -> three Wc_d weight matrices
  - vhat2   = (W2 diag(cc)) @ (z1*s1) -> streamed g1c with plain W2
  - kf = (gs*f+gb)^2 squared on host; AC/BC folded into head weights
Head:
  m1 = (diag(AC,BC) W4) z3 + hb4 ;  c2 = (diag(AC,BC)(-2 W4)) v3
  host: res = c2 + kf*m1

Engine balance (the point of this version): the baseline saturated the
scalar engine (~25us/tile of ACTIVATEs) while GpSimd idled.  Here every
PSUM tile has exactly ONE reader, the elementwise jet chain is spread
across ACT / DVE / GpSimd (~17us/tile each), fused custom DVE ops
(SQSQ/ADDSQ) halve the sum-of-squares chains, and z3*q3 is accumulated
into the pv3 PSUM group via an identity matmul so the layer-3 v-channel
needs no separate PSUM-add pass.  Layer-3 tail (pv3 + head) of tile j is
emitted inside tile j+1's instruction stream so the PE never waits on the
elementwise chain.

Sharding: pure data parallel over 8 NeuronCores (16384 points each),
weights replicated, no collectives.
"""

import math
from contextlib import ExitStack

import numpy as np

import concourse.bass as bass
import concourse.bacc as bacc
import concourse.mybir as mybir
import concourse.tile as tile
import concourse.bass_utils as bass_utils
from concourse.bass_utils import run_bass_kernel_spmd

from concourse import dve_ops as _DO
from concourse.dve_spec import Spec as _Spec, Src0 as _Src0, Src1 as _Src1, \
    One as _One, lower as _lower
from concourse.dve_uop import DveOpSpec as _DveOpSpec


def _register_op(name, spec, subdim=False):
    if name in _DO._SUB_OPCODE_FOR_NAME:
        for op in _DO.OPS:
            if op.name == name:
                return op
    shas = {}
    opcode = max(_DO._SUB_OPCODE_FOR_NAME.values()) + 1
    for ver in ("v3", "v4"):
        try:
            uops = _lower(spec, ver=ver)
        except Exception:
            continue
        tmp = _DveOpSpec(name=name, opcode=opcode, uops=uops,
                         rd1_en=_DO.has_src1(spec))
        shas[ver] = tmp.sha(ver)
    op = _DO.DveOp(name, spec, subdim, shas)
    _DO.OPS.append(op)
    _DO._SUB_OPCODE_FOR_NAME[name] = opcode
    return op


# v = (1 - z^2) * w   (Src0 = z, Src1 = w)
BVP_SMUL = _register_op(
    "BVP_SMUL_ANT",
    _Spec(body=(_One - _Src0 * _Src0) * _Src1,
          reference=lambda in0, in1, c0, c1, c2: (1.0 - in0 * in0) * in1),
)

# q = a^2 + b^2
BVP_SQSQ = _register_op(
    "BVP_SQSQ_ANT",
    _Spec(body=_Src0 * _Src0 + _Src1 * _Src1,
          reference=lambda in0, in1, c0, c1, c2: in0 * in0 + in1 * in1),
)

# q = a + b^2
BVP_ADDSQ = _register_op(
    "BVP_ADDSQ_ANT",
    _Spec(body=_Src0 + _Src1 * _Src1,
          reference=lambda in0, in1, c0, c1, c2: in0 + in1 * in1),
)

# ---- problem constants (from the BVPModel definition) ----
_C0 = 343.0
FC, F0 = 500.0, 100.0
XC, YC, ZC = 0.7, 0.5, 0.6
AC, A0 = 2.0, 0.1
BC, B0 = 1.5, -0.05
BETA = ((YC * ZC) ** 2, (XC * ZC) ** 2, (XC * YC) ** 2)

_GS = np.float32(2.0 * math.pi * FC * (XC * YC * ZC) / _C0)
_GB = np.float32(2.0 * math.pi * F0 * (XC * YC * ZC) / _C0)

N_TOTAL = 131072
N_CORES = 8
NPC = N_TOTAL // N_CORES  # 16384 points per core
H = 256
B = 1024                  # points per tile
NT = NPC // B             # tiles per core

F32 = mybir.dt.float32
F16 = mybir.dt.float16

Alu = mybir.AluOpType
Act = mybir.ActivationFunctionType


def _build_program():
    """Build the per-core Bass program (identical on all 8 cores)."""
    nc = bacc.Bacc("TRN2", target_bir_lowering=False, debug=False)

    # ---- DRAM I/O ----
    d_z1 = nc.dram_tensor("z1", [H, NPC], F16, kind="ExternalInput").ap()
    d_s1 = nc.dram_tensor("s1", [H, NPC], F16, kind="ExternalInput").ap()
    d_g1c = nc.dram_tensor("g1c", [H, NPC], F16, kind="ExternalInput").ap()
    d_w2t = nc.dram_tensor("w2t", [H, H], F16, kind="ExternalInput").ap()
    d_w3t = nc.dram_tensor("w3t", [H, H], F16, kind="ExternalInput").ap()
    d_wct = [
        nc.dram_tensor(f"wct{d}", [H, H], F16, kind="ExternalInput").ap()
        for d in range(3)
    ]
    d_whz = nc.dram_tensor("whz", [H, 2], F16, kind="ExternalInput").ap()
    d_whv = nc.dram_tensor("whv", [H, 2], F16, kind="ExternalInput").ap()
    d_bias = nc.dram_tensor("bias", [H, 2], F32, kind="ExternalInput").ap()
    d_hbias = nc.dram_tensor("hbias", [34, 1], F32, kind="ExternalInput").ap()
    d_ident = nc.dram_tensor("ident", [128, 128], F16,
                             kind="ExternalInput").ap()
    d_out = nc.dram_tensor("out", [4, NPC], F16, kind="ExternalOutput").ap()

    with tile.TileContext(nc) as tc, ExitStack() as ctx:
        singles = ctx.enter_context(tc.tile_pool(name="singles", bufs=1))
        work = ctx.enter_context(tc.tile_pool(name="work", bufs=1))
        psum = ctx.enter_context(tc.tile_pool(name="psum", bufs=1,
                                              space="PSUM"))

        # ---- load weights / constants once ----
        w2t = [singles.tile([128, H], F16, name=f"w2t{k}") for k in range(2)]
        w3t = [singles.tile([128, H], F16, name=f"w3t{k}") for k in range(2)]
        wct = [[singles.tile([128, H], F16, name=f"wct{d}_{k}")
                for k in range(2)] for d in range(3)]
        whz = [singles.tile([128, 2], F16, name=f"whz{k}") for k in range(2)]
        whv = [singles.tile([128, 2], F16, name=f"whv{k}") for k in range(2)]
        bias = [singles.tile([128, 2], F32, name=f"bias{k}") for k in range(2)]
        for k in range(2):
            sl = slice(k * 128, (k + 1) * 128)
            nc.sync.dma_start(out=w2t[k], in_=d_w2t[sl, :])
            nc.sync.dma_start(out=w3t[k], in_=d_w3t[sl, :])
            for d in range(3):
                nc.sync.dma_start(out=wct[d][k], in_=d_wct[d][sl, :])
            nc.sync.dma_start(out=whz[k], in_=d_whz[sl, :])
            nc.sync.dma_start(out=whv[k], in_=d_whv[sl, :])
            nc.sync.dma_start(out=bias[k], in_=d_bias[sl, :])
        hbias = singles.tile([34, 1], F32)
        nc.sync.dma_start(out=hbias, in_=d_hbias)
        ident = singles.tile([128, 128], F16)
        nc.sync.dma_start(out=ident, in_=d_ident)

        HS = [slice(0, 512), slice(512, 1024)]

        def emit_finish(p):
            """Layer-3 v-channel tail + head for a previous tile.

            pv3 = W3 @ v2 (+ I @ t3, accumulated on the PE) per 512-chunk,
            then v3 = (1 - z3^2) * pv3 and the packed 2-col-group head.
            """
            j, z3, v2, t3 = p["j"], p["z3"], p["v2"], p["t3"]
            js = slice(j * B, (j + 1) * B)
            v3 = [work.tile([128, B], F16, name=f"v3_{m}", bufs=2)
                  for m in range(2)]
            for m in range(2):
                msl = slice(m * 128, (m + 1) * 128)
                pz = psum.tile([128, B], F32, tag="pmain", bufs=3,
                               name=f"pv3_{m}")
                for c in range(2):
                    hs = HS[c]
                    nc.tensor.matmul(pz[:, hs], w3t[0][:, msl], v2[0][:, hs],
                                     start=True, stop=False)
                    nc.tensor.matmul(pz[:, hs], w3t[1][:, msl], v2[1][:, hs],
                                     start=False, stop=False)
                    nc.tensor.matmul(pz[:, hs], ident, t3[m][:, hs],
                                     start=False, stop=True)
                nc.vector._custom_dve(BVP_SMUL, out=v3[m], in0=z3[m], in1=pz)
            hout = work.tile([34, B], F16, name="hout", bufs=2)
            ph34 = psum.tile([34, B], F32, tag="pmain", bufs=3, name="ph34")
            for c in range(2):
                hs = HS[c]
                for k in range(2):
                    nc.tensor.matmul(ph34[0:2, hs], whz[k], z3[k][:, hs],
                                     start=(k == 0), stop=(k == 1))
                for k in range(2):
                    nc.tensor.matmul(ph34[32:34, hs], whv[k], v3[k][:, hs],
                                     start=(k == 0), stop=(k == 1))
            nc.scalar.activation(hout, ph34, Act.Identity, bias=hbias)
            nc.sync.dma_start(out=d_out[0:2, js], in_=hout[0:2, :])
            nc.sync.dma_start(out=d_out[2:4, js], in_=hout[32:34, :])

        prev = None
        for j in range(NT):
            js = slice(j * B, (j + 1) * B)

            # ---------- stream layer-1 jet (host-computed) ----------
            z1, s1, g1 = [], [], []
            for k in range(2):
                sl = slice(k * 128, (k + 1) * 128)
                z = work.tile([128, B], F16, name=f"z1_{k}", bufs=2)
                nc.sync.dma_start(out=z, in_=d_z1[sl, js])
                s = work.tile([128, B], F16, name=f"s1_{k}", bufs=2)
                nc.sync.dma_start(out=s, in_=d_s1[sl, js])
                g = work.tile([128, B], F16, name=f"g1_{k}", bufs=2)
                nc.sync.dma_start(out=g, in_=d_g1c[sl, js])
                z1.append(z)
                s1.append(s)
                g1.append(g)

            # ---------- layer 2, pass 1: h-channel both halves ----------
            z2, s2 = [], []
            for m in range(2):
                msl = slice(m * 128, (m + 1) * 128)
                ph = psum.tile([128, B], F32, tag="pmain", bufs=3,
                               name=f"ph2_{m}")
                for k in range(2):
                    for c in range(2):
                        nc.tensor.matmul(ph[:, HS[c]], w2t[k][:, msl],
                                         z1[k][:, HS[c]],
                                         start=(k == 0), stop=(k == 1))
                z = work.tile([128, B], F16, name=f"z2_{m}")
                nc.scalar.activation(z, ph, Act.Tanh, bias=bias[m][:, 0:1])
                zq = work.tile([128, B], F16, name=f"zq2_{m}")
                nc.vector.tensor_mul(zq, z, z)
                s = work.tile([128, B], F16, name=f"s2_{m}")
                nc.vector.tensor_scalar(s, zq, -1.0, 1.0, Alu.mult, Alu.add)
                z2.append(z)
                s2.append(s)

            # ---------- layer 2, pass 2: tangent channels ----------
            # u-muls for BOTH halves precede the q-chains in the DVE FIFO:
            # u2 gates layer-3 MMs, while the q-chain output (tl2) is only
            # consumed by the deferred pv2/STT block below.
            u2, tl2l, uhs = [[None] * 2 for _ in range(3)], [], []
            for m in range(2):
                msl = slice(m * 128, (m + 1) * 128)
                uh = []
                for d in range(3):
                    pu = psum.tile([128, B], F32, tag="pmain", bufs=3,
                                   name=f"pu2_{d}{m}")
                    for k in range(2):
                        for c in range(2):
                            nc.tensor.matmul(pu[:, HS[c]], wct[d][k][:, msl],
                                             s1[k][:, HS[c]],
                                             start=(k == 0), stop=(k == 1))
                    t = work.tile([128, B], F16, name=f"uh2_{d}{m}")
                    nc.scalar.activation(t, pu, Act.Copy)
                    uh.append(t)
                for d in range(3):
                    u = work.tile([128, B], F16, name=f"u2_{d}{m}")
                    nc.vector.tensor_mul(u, uh[d], s2[m])
                    u2[d][m] = u
                uhs.append(uh)
            for m in range(2):
                uh = uhs[m]
                q1 = work.tile([128, B], F16, name=f"q1l2_{m}")
                nc.vector._custom_dve(BVP_SQSQ, out=q1, in0=uh[0], in1=uh[1])
                q2 = work.tile([128, B], F16, name=f"q2l2_{m}")
                nc.vector._custom_dve(BVP_ADDSQ, out=q2, in0=q1, in1=uh[2])
                tl2 = work.tile([128, B], F16, name=f"tl2_{m}")
                nc.vector.tensor_mul(tl2, q2, z2[m])
                tl2l.append(tl2)

            # ---------- layer 3 (h + tangents; v-tail deferred) ----------
            z3, t3 = [], []
            for m in range(2):
                msl = slice(m * 128, (m + 1) * 128)
                ph = psum.tile([128, B], F32, tag="pmain", bufs=3,
                               name=f"ph3_{m}")
                pu = [psum.tile([128, B], F32, tag="pmain", bufs=3,
                                name=f"pu3_{d}{m}") for d in range(3)]
                for k in range(2):
                    for c in range(2):
                        nc.tensor.matmul(ph[:, HS[c]], w3t[k][:, msl],
                                         z2[k][:, HS[c]],
                                         start=(k == 0), stop=(k == 1))
                    for d in range(3):
                        for c in range(2):
                            nc.tensor.matmul(pu[d][:, HS[c]],
                                             w3t[k][:, msl],
                                             u2[d][k][:, HS[c]],
                                             start=(k == 0), stop=(k == 1))
                z = work.tile([128, B], F16, name=f"z3_{m}", bufs=2)
                nc.scalar.activation(z, ph, Act.Tanh, bias=bias[m][:, 1:2])
                ysx = work.tile([128, B], F16, name=f"ysx_{m}")
                nc.scalar.activation(ysx, pu[0], Act.Square)
                ysy = work.tile([128, B], F16, name=f"ysy_{m}")
                nc.scalar.activation(ysy, pu[1], Act.Square)
                q13 = work.tile([128, B], F16, name=f"q13_{m}")
                nc.vector.tensor_add(q13, ysx, ysy)
                q3 = work.tile([128, B], F16, name=f"q3_{m}")
                nc.vector._custom_dve(BVP_ADDSQ, out=q3, in0=q13, in1=pu[2])
                t = work.tile([128, B], F16, name=f"t3_{m}", bufs=2)
                nc.gpsimd.tensor_mul(t, q3, z)
                z3.append(z)
                t3.append(t)

            # ---------- layer-3 tail + head of the previous tile ----------
            if prev is not None:
                emit_finish(prev)

            # ---------- layer-2 v-channel (input-only; emitted LAST so its
            # PSUM slots — held until the DVE STT fires — recycle into the
            # next tile's ph2 allocations rather than blocking pv3/head.
            # v2 itself is not needed until the NEXT tile's finish block) ---
            v2 = []
            for m in range(2):
                msl = slice(m * 128, (m + 1) * 128)
                pv = psum.tile([128, B], F32, tag="pmain", bufs=3,
                               name=f"pv2_{m}")
                for k in range(2):
                    for c in range(2):
                        nc.tensor.matmul(pv[:, HS[c]], w2t[k][:, msl],
                                         g1[k][:, HS[c]],
                                         start=(k == 0), stop=(k == 1))
                t2 = work.tile([128, B], F16, name=f"t2l2_{m}")
                nc.vector.scalar_tensor_tensor(t2, tl2l[m], 1.0, pv,
                                               op0=Alu.mult, op1=Alu.add)
                v = work.tile([128, B], F16, name=f"v2_{m}", bufs=2)
                nc.gpsimd.tensor_mul(v, t2, s2[m])
                v2.append(v)

            prev = {"j": j, "z3": z3, "v2": v2, "t3": t3}

        emit_finish(prev)

    nc.compile()
    return nc


def _host_prep(inputs):
    """Host-side: layer-1 forward jet + weight folds + sharding."""
    x = np.asarray(inputs["x"], np.float32)
    y = np.asarray(inputs["y"], np.float32)
    z = np.asarray(inputs["z"], np.float32)
    f = np.asarray(inputs["f"], np.float32)
    W1 = np.asarray(inputs["W1"], np.float32)
    b1 = np.asarray(inputs["b1"], np.float32)
    W2 = np.asarray(inputs["W2"], np.float32)
    b2 = np.asarray(inputs["b2"], np.float32)
    W3 = np.asarray(inputs["W3"], np.float32)
    b3 = np.asarray(inputs["b3"], np.float32)
    W4 = np.asarray(inputs["W4"], np.float32)
    b4 = np.asarray(inputs["b4"], np.float32)

    sb = np.sqrt(np.asarray(BETA, np.float64))
    chat = (sb[None, :] * W1[:, :3].astype(np.float64)).astype(np.float32)
    cc = (np.asarray(BETA)[None, :] * W1[:, :3].astype(np.float64) ** 2) \
        .sum(1).astype(np.float32)                              # [256]

    xyzf = np.stack([x, y, z, f])                               # [4, N] f32
    a1 = W1 @ xyzf + b1[:, None]
    z1f = np.tanh(a1)
    s1f = (1.0 - z1f * z1f).astype(np.float32)
    g1cf = (cc[:, None] * z1f * s1f).astype(np.float32)

    z1 = z1f.astype(np.float16)
    s1 = s1f.astype(np.float16)
    g1c = g1cf.astype(np.float16)

    biasm = np.ascontiguousarray(np.stack([b2, b3], axis=1))    # [256, 2]
    w2t = np.ascontiguousarray(W2.T.astype(np.float16))
    w3t = np.ascontiguousarray(W3.T.astype(np.float16))
    wct = [
        np.ascontiguousarray((W2 * chat[None, :, d]).T.astype(np.float16))
        for d in range(3)
    ]
    ab = np.array([AC, BC], np.float32)
    whz = np.ascontiguousarray((ab[:, None] * W4).T.astype(np.float16))
    whv = np.ascontiguousarray((ab[:, None] * (-2.0 * W4)).T
                               .astype(np.float16))
    hbias = np.zeros((34, 1), np.float32)
    hbias[0, 0] = AC * b4[0] + A0
    hbias[1, 0] = BC * b4[1] + B0
    identm = np.eye(128, dtype=np.float16)

    in_maps = []
    for c in range(N_CORES):
        cs = slice(c * NPC, (c + 1) * NPC)
        im = {
            "z1": np.ascontiguousarray(z1[:, cs]),
            "s1": np.ascontiguousarray(s1[:, cs]),
            "g1c": np.ascontiguousarray(g1c[:, cs]),
            "w2t": w2t, "w3t": w3t,
            "whz": whz, "whv": whv,
            "bias": biasm, "hbias": hbias, "ident": identm,
        }
        for d in range(3):
            im[f"wct{d}"] = wct[d]
        in_maps.append(im)
    return in_maps


_NC_CACHE = None


def get_program():
    global _NC_CACHE
    if _NC_CACHE is None:
        _NC_CACHE = _build_program()
    return _NC_CACHE


def kernel(**inputs) -> np.ndarray:
    nc = get_program()
    in_maps = _host_prep(inputs)
    r = run_bass_kernel_spmd(nc, in_maps, core_ids=list(range(N_CORES)))
    out4 = np.concatenate(
        [r.results[c]["out"] for c in range(N_CORES)], axis=1
    ).astype(np.float32)
    f = np.asarray(inputs["f"], np.float64)
    kf = ((_GS * f + _GB) ** 2).astype(np.float32)
    return out4[2:4] + kf[None, :] * out4[0:2]


# revision 20
# speedup vs baseline: 1.1905x; 1.1905x over previous
"""Trainium2 Bass kernel for the BVPModel Helmholtz-residual PINN problem.

Computes, for N=131072 collocation points, the residual of a Helmholtz-type
PDE through a 4-256-256-256-2 tanh MLP, including the beta-weighted sum of
diagonal second derivatives w.r.t. the three spatial inputs.

Math: forward-mode second-order jets (forward Laplacian). Per point:
  - primal       h_l
  - tangents     u_l^d (scaled by sqrt(beta_d), d=x,y,z)
  - second-order v_l   (beta-weighted; sign/scale folded into head)
through the layers:
  affine:  hat = W h,   uhat_d = W u_d,   vhat = W v
  tanh:    z = tanh(hat+b), s = 1 - z^2
           u_d = s * uhat_d
           v   = (z * sum_d uhat_d^2 + vhat) * s
Folds (all host-side, free on device):
  - a1 = W1 @ [x,y,z,f] + b1 computed on host, streamed in as fp16
  - uhat2_d = (W2 diag(chat_d)) @ s1  -> three Wc_d weight matrices
  - vhat2   = (W2 diag(cc)) @ (z1*s1) -> streamed g1c with plain W2
  - kf = (gs*f+gb)^2 squared on host; AC/BC folded into head weights
Head:
  m1 = (diag(AC,BC) W4) z3 + hb4 ;  c2 = (diag(AC,BC)(-2 W4)) v3
  host: res = c2 + kf*m1

Engine balance (the point of this version): the baseline saturated the
scalar engine (~25us/tile of ACTIVATEs) while GpSimd idled.  Here every
PSUM tile has exactly ONE reader, the elementwise jet chain is spread
across ACT / DVE / GpSimd (~17us/tile each), fused custom DVE ops
(SQSQ/ADDSQ) halve the sum-of-squares chains, and z3*q3 is accumulated
into the pv3 PSUM group via an identity matmul so the layer-3 v-channel
needs no separate PSUM-add pass.  Layer-3 tail (pv3 + head) of tile j is
emitted inside tile j+1's instruction stream so the PE never waits on the
elementwise chain.

Sharding: pure data parallel over 8 NeuronCores (16384 points each),
weights replicated, no collectives.
"""

import math
from contextlib import ExitStack

import numpy as np

import concourse.bass as bass
import concourse.bacc as bacc
import concourse.mybir as mybir
import concourse.tile as tile
import concourse.bass_utils as bass_utils
from concourse.bass_utils import run_bass_kernel_spmd

from concourse import dve_ops as _DO
from concourse.dve_spec import Spec as _Spec, Src0 as _Src0, Src1 as _Src1, \
    One as _One, lower as _lower
from concourse.dve_uop import DveOpSpec as _DveOpSpec


def _register_op(name, spec, subdim=False):
    if name in _DO._SUB_OPCODE_FOR_NAME:
        for op in _DO.OPS:
            if op.name == name:
                return op
    shas = {}
    opcode = max(_DO._SUB_OPCODE_FOR_NAME.values()) + 1
    for ver in ("v3", "v4"):
        try:
            uops = _lower(spec, ver=ver)
        except Exception:
            continue
        tmp = _DveOpSpec(name=name, opcode=opcode, uops=uops,
                         rd1_en=_DO.has_src1(spec))
        shas[ver] = tmp.sha(ver)
    op = _DO.DveOp(name, spec, subdim, shas)
    _DO.OPS.append(op)
    _DO._SUB_OPCODE_FOR_NAME[name] = opcode
    return op


# v = (1 - z^2) * w   (Src0 = z, Src1 = w)
BVP_SMUL = _register_op(
    "BVP_SMUL_ANT",
    _Spec(body=(_One - _Src0 * _Src0) * _Src1,
          reference=lambda in0, in1, c0, c1, c2: (1.0 - in0 * in0) * in1),
)

# q = a^2 + b^2
BVP_SQSQ = _register_op(
    "BVP_SQSQ_ANT",
    _Spec(body=_Src0 * _Src0 + _Src1 * _Src1,
          reference=lambda in0, in1, c0, c1, c2: in0 * in0 + in1 * in1),
)

# q = a + b^2
BVP_ADDSQ = _register_op(
    "BVP_ADDSQ_ANT",
    _Spec(body=_Src0 + _Src1 * _Src1,
          reference=lambda in0, in1, c0, c1, c2: in0 + in1 * in1),
)

# ---- problem constants (from the BVPModel definition) ----
_C0 = 343.0
FC, F0 = 500.0, 100.0
XC, YC, ZC = 0.7, 0.5, 0.6
AC, A0 = 2.0, 0.1
BC, B0 = 1.5, -0.05
BETA = ((YC * ZC) ** 2, (XC * ZC) ** 2, (XC * YC) ** 2)

_GS = np.float32(2.0 * math.pi * FC * (XC * YC * ZC) / _C0)
_GB = np.float32(2.0 * math.pi * F0 * (XC * YC * ZC) / _C0)

N_TOTAL = 131072
N_CORES = 8
NPC = N_TOTAL // N_CORES  # 16384 points per core
H = 256
B = 1024                  # points per tile
NT = NPC // B             # tiles per core

F32 = mybir.dt.float32
F16 = mybir.dt.float16

Alu = mybir.AluOpType
Act = mybir.ActivationFunctionType


def _build_program():
    """Build the per-core Bass program (identical on all 8 cores)."""
    nc = bacc.Bacc("TRN2", target_bir_lowering=False, debug=False)

    # ---- DRAM I/O ----
    d_z1 = nc.dram_tensor("z1", [H, NPC], F16, kind="ExternalInput").ap()
    d_s1 = nc.dram_tensor("s1", [H, NPC], F16, kind="ExternalInput").ap()
    d_g1c = nc.dram_tensor("g1c", [H, NPC], F16, kind="ExternalInput").ap()
    d_w2t = nc.dram_tensor("w2t", [H, H], F16, kind="ExternalInput").ap()
    d_w3t = nc.dram_tensor("w3t", [H, H], F16, kind="ExternalInput").ap()
    d_wct = [
        nc.dram_tensor(f"wct{d}", [H, H], F16, kind="ExternalInput").ap()
        for d in range(3)
    ]
    d_whz = nc.dram_tensor("whz", [H, 2], F16, kind="ExternalInput").ap()
    d_whv = nc.dram_tensor("whv", [H, 2], F16, kind="ExternalInput").ap()
    d_bias = nc.dram_tensor("bias", [H, 2], F32, kind="ExternalInput").ap()
    d_hbias = nc.dram_tensor("hbias", [34, 1], F32, kind="ExternalInput").ap()
    d_ident = nc.dram_tensor("ident", [128, 128], F16,
                             kind="ExternalInput").ap()
    d_out = nc.dram_tensor("out", [4, NPC], F16, kind="ExternalOutput").ap()

    with tile.TileContext(nc) as tc, ExitStack() as ctx:
        singles = ctx.enter_context(tc.tile_pool(name="singles", bufs=1))
        work = ctx.enter_context(tc.tile_pool(name="work", bufs=1))
        psum = ctx.enter_context(tc.tile_pool(name="psum", bufs=1,
                                              space="PSUM"))

        # ---- load weights / constants once ----
        w2t = [singles.tile([128, H], F16, name=f"w2t{k}") for k in range(2)]
        w3t = [singles.tile([128, H], F16, name=f"w3t{k}") for k in range(2)]
        wct = [[singles.tile([128, H], F16, name=f"wct{d}_{k}")
                for k in range(2)] for d in range(3)]
        whz = [singles.tile([128, 2], F16, name=f"whz{k}") for k in range(2)]
        whv = [singles.tile([128, 2], F16, name=f"whv{k}") for k in range(2)]
        bias = [singles.tile([128, 2], F32, name=f"bias{k}") for k in range(2)]
        for k in range(2):
            sl = slice(k * 128, (k + 1) * 128)
            nc.sync.dma_start(out=w2t[k], in_=d_w2t[sl, :])
            nc.sync.dma_start(out=w3t[k], in_=d_w3t[sl, :])
            for d in range(3):
                nc.sync.dma_start(out=wct[d][k], in_=d_wct[d][sl, :])
            nc.sync.dma_start(out=whz[k], in_=d_whz[sl, :])
            nc.sync.dma_start(out=whv[k], in_=d_whv[sl, :])
            nc.sync.dma_start(out=bias[k], in_=d_bias[sl, :])
        hbias = singles.tile([34, 1], F32)
        nc.sync.dma_start(out=hbias, in_=d_hbias)
        ident = singles.tile([128, 128], F16)
        nc.sync.dma_start(out=ident, in_=d_ident)

        HS = [slice(0, 512), slice(512, 1024)]

        def emit_finish(p):
            """Layer-3 v-channel tail + head for a previous tile.

            pv3 = W3 @ v2 (+ I @ t3, accumulated on the PE) per 512-chunk,
            then v3 = (1 - z3^2) * pv3 and the packed 2-col-group head.
            """
            j, z3, v2, t3 = p["j"], p["z3"], p["v2"], p["t3"]
            js = slice(j * B, (j + 1) * B)
            v3 = [work.tile([128, B], F16, name=f"v3_{m}", bufs=2)
                  for m in range(2)]
            for m in range(2):
                msl = slice(m * 128, (m + 1) * 128)
                pz = psum.tile([128, B], F32, tag="pmain", bufs=3,
                               name=f"pv3_{m}")
                for c in range(2):
                    hs = HS[c]
                    nc.tensor.matmul(pz[:, hs], w3t[0][:, msl], v2[0][:, hs],
                                     start=True, stop=False)
                    nc.tensor.matmul(pz[:, hs], w3t[1][:, msl], v2[1][:, hs],
                                     start=False, stop=False)
                    nc.tensor.matmul(pz[:, hs], ident, t3[m][:, hs],
                                     start=False, stop=True)
                nc.vector._custom_dve(BVP_SMUL, out=v3[m], in0=z3[m], in1=pz)
            hout = work.tile([34, B], F16, name="hout", bufs=2)
            ph34 = psum.tile([34, B], F32, tag="pmain", bufs=3, name="ph34")
            for c in range(2):
                hs = HS[c]
                for k in range(2):
                    nc.tensor.matmul(ph34[0:2, hs], whz[k], z3[k][:, hs],
                                     start=(k == 0), stop=(k == 1))
                for k in range(2):
                    nc.tensor.matmul(ph34[32:34, hs], whv[k], v3[k][:, hs],
                                     start=(k == 0), stop=(k == 1))
            nc.scalar.activation(hout, ph34, Act.Identity, bias=hbias)
            nc.sync.dma_start(out=d_out[0:2, js], in_=hout[0:2, :])
            nc.sync.dma_start(out=d_out[2:4, js], in_=hout[32:34, :])

        prev = None
        for j in range(NT):
            js = slice(j * B, (j + 1) * B)

            # ---------- stream layer-1 jet (host-computed) ----------
            z1, s1, g1 = [], [], []
            for k in range(2):
                sl = slice(k * 128, (k + 1) * 128)
                z = work.tile([128, B], F16, name=f"z1_{k}", bufs=2)
                nc.sync.dma_start(out=z, in_=d_z1[sl, js])
                s = work.tile([128, B], F16, name=f"s1_{k}", bufs=2)
                nc.sync.dma_start(out=s, in_=d_s1[sl, js])
                g = work.tile([128, B], F16, name=f"g1_{k}", bufs=2)
                nc.sync.dma_start(out=g, in_=d_g1c[sl, js])
                z1.append(z)
                s1.append(s)
                g1.append(g)

            # ---------- layer 2, pass 1: h-channel both halves ----------
            z2, s2 = [], []
            for m in range(2):
                msl = slice(m * 128, (m + 1) * 128)
                ph = psum.tile([128, B], F32, tag="pmain", bufs=3,
                               name=f"ph2_{m}")
                for k in range(2):
                    for c in range(2):
                        nc.tensor.matmul(ph[:, HS[c]], w2t[k][:, msl],
                                         z1[k][:, HS[c]],
                                         start=(k == 0), stop=(k == 1))
                z = work.tile([128, B], F16, name=f"z2_{m}")
                nc.scalar.activation(z, ph, Act.Tanh, bias=bias[m][:, 0:1])
                zq = work.tile([128, B], F16, name=f"zq2_{m}")
                nc.vector.tensor_mul(zq, z, z)
                s = work.tile([128, B], F16, name=f"s2_{m}")
                nc.vector.tensor_scalar(s, zq, -1.0, 1.0, Alu.mult, Alu.add)
                z2.append(z)
                s2.append(s)

            # ---------- layer 2, pass 2: tangent channels ----------
            # u-muls for BOTH halves precede the q-chains in the DVE FIFO:
            # u2 gates layer-3 MMs, while the q-chain output (tl2) is only
            # consumed by the deferred pv2/STT block below.
            u2, tl2l, uhs = [[None] * 2 for _ in range(3)], [], []
            for m in range(2):
                msl = slice(m * 128, (m + 1) * 128)
                uh = []
                for d in range(3):
                    pu = psum.tile([128, B], F32, tag="pmain", bufs=3,
                                   name=f"pu2_{d}{m}")
                    for k in range(2):
                        for c in range(2):
                            nc.tensor.matmul(pu[:, HS[c]], wct[d][k][:, msl],
                                             s1[k][:, HS[c]],
                                             start=(k == 0), stop=(k == 1))
                    t = work.tile([128, B], F16, name=f"uh2_{d}{m}")
                    nc.scalar.activation(t, pu, Act.Copy)
                    uh.append(t)
                for d in range(3):
                    u = work.tile([128, B], F16, name=f"u2_{d}{m}")
                    nc.vector.tensor_mul(u, uh[d], s2[m])
                    u2[d][m] = u
                uhs.append(uh)
            for m in range(2):
                uh = uhs[m]
                q1 = work.tile([128, B], F16, name=f"q1l2_{m}")
                nc.vector._custom_dve(BVP_SQSQ, out=q1, in0=uh[0], in1=uh[1])
                q2 = work.tile([128, B], F16, name=f"q2l2_{m}")
                nc.vector._custom_dve(BVP_ADDSQ, out=q2, in0=q1, in1=uh[2])
                tl2 = work.tile([128, B], F16, name=f"tl2_{m}")
                nc.vector.tensor_mul(tl2, q2, z2[m])
                tl2l.append(tl2)

            # ---------- layer 3 (h + tangents; v-tail deferred) ----------
            z3, t3 = [], []
            for m in range(2):
                msl = slice(m * 128, (m + 1) * 128)
                ph = psum.tile([128, B], F32, tag="pmain", bufs=3,
                               name=f"ph3_{m}")
                pu = [psum.tile([128, B], F32, tag="pmain", bufs=3,
                                name=f"pu3_{d}{m}") for d in range(3)]
                for k in range(2):
                    for c in range(2):
                        nc.tensor.matmul(ph[:, HS[c]], w3t[k][:, msl],
                                         z2[k][:, HS[c]],
                                         start=(k == 0), stop=(k == 1))
                    for d in range(3):
                        for c in range(2):
                            nc.tensor.matmul(pu[d][:, HS[c]],
                                             w3t[k][:, msl],
                                             u2[d][k][:, HS[c]],
                                             start=(k == 0), stop=(k == 1))
                z = work.tile([128, B], F16, name=f"z3_{m}", bufs=2)
                nc.scalar.activation(z, ph, Act.Tanh, bias=bias[m][:, 1:2])
                ysx = work.tile([128, B], F16, name=f"ysx_{m}")
                nc.scalar.activation(ysx, pu[0], Act.Square)
                ysy = work.tile([128, B], F16, name=f"ysy_{m}")
                nc.scalar.activation(ysy, pu[1], Act.Square)
                ysz = work.tile([128, B], F16, name=f"ysz_{m}")
                nc.scalar.activation(ysz, pu[2], Act.Square)
                q13 = work.tile([128, B], F16, name=f"q13_{m}")
                nc.vector.tensor_add(q13, ysx, ysy)
                q3 = work.tile([128, B], F16, name=f"q3_{m}")
                nc.vector.tensor_add(q3, q13, ysz)
                t = work.tile([128, B], F16, name=f"t3_{m}", bufs=2)
                nc.gpsimd.tensor_mul(t, q3, z)
                z3.append(z)
                t3.append(t)

            # ---------- layer-3 tail + head of the previous tile ----------
            if prev is not None:
                emit_finish(prev)

            # ---------- layer-2 v-channel (input-only; emitted LAST so its
            # PSUM slots — held until the DVE STT fires — recycle into the
            # next tile's ph2 allocations rather than blocking pv3/head.
            # v2 itself is not needed until the NEXT tile's finish block) ---
            v2 = []
            for m in range(2):
                msl = slice(m * 128, (m + 1) * 128)
                pv = psum.tile([128, B], F32, tag="pmain", bufs=3,
                               name=f"pv2_{m}")
                for k in range(2):
                    for c in range(2):
                        nc.tensor.matmul(pv[:, HS[c]], w2t[k][:, msl],
                                         g1[k][:, HS[c]],
                                         start=(k == 0), stop=(k == 1))
                t2 = work.tile([128, B], F16, name=f"t2l2_{m}")
                nc.vector.scalar_tensor_tensor(t2, tl2l[m], 1.0, pv,
                                               op0=Alu.mult, op1=Alu.add)
                v = work.tile([128, B], F16, name=f"v2_{m}", bufs=2)
                nc.gpsimd.tensor_mul(v, t2, s2[m])
                v2.append(v)

            prev = {"j": j, "z3": z3, "v2": v2, "t3": t3}

        emit_finish(prev)

    nc.compile()
    return nc


def _host_prep(inputs):
    """Host-side: layer-1 forward jet + weight folds + sharding."""
    x = np.asarray(inputs["x"], np.float32)
    y = np.asarray(inputs["y"], np.float32)
    z = np.asarray(inputs["z"], np.float32)
    f = np.asarray(inputs["f"], np.float32)
    W1 = np.asarray(inputs["W1"], np.float32)
    b1 = np.asarray(inputs["b1"], np.float32)
    W2 = np.asarray(inputs["W2"], np.float32)
    b2 = np.asarray(inputs["b2"], np.float32)
    W3 = np.asarray(inputs["W3"], np.float32)
    b3 = np.asarray(inputs["b3"], np.float32)
    W4 = np.asarray(inputs["W4"], np.float32)
    b4 = np.asarray(inputs["b4"], np.float32)

    sb = np.sqrt(np.asarray(BETA, np.float64))
    chat = (sb[None, :] * W1[:, :3].astype(np.float64)).astype(np.float32)
    cc = (np.asarray(BETA)[None, :] * W1[:, :3].astype(np.float64) ** 2) \
        .sum(1).astype(np.float32)                              # [256]

    xyzf = np.stack([x, y, z, f])                               # [4, N] f32
    a1 = W1 @ xyzf + b1[:, None]
    z1f = np.tanh(a1)
    s1f = (1.0 - z1f * z1f).astype(np.float32)
    g1cf = (cc[:, None] * z1f * s1f).astype(np.float32)

    z1 = z1f.astype(np.float16)
    s1 = s1f.astype(np.float16)
    g1c = g1cf.astype(np.float16)

    biasm = np.ascontiguousarray(np.stack([b2, b3], axis=1))    # [256, 2]
    w2t = np.ascontiguousarray(W2.T.astype(np.float16))
    w3t = np.ascontiguousarray(W3.T.astype(np.float16))
    wct = [
        np.ascontiguousarray((W2 * chat[None, :, d]).T.astype(np.float16))
        for d in range(3)
    ]
    ab = np.array([AC, BC], np.float32)
    whz = np.ascontiguousarray((ab[:, None] * W4).T.astype(np.float16))
    whv = np.ascontiguousarray((ab[:, None] * (-2.0 * W4)).T
                               .astype(np.float16))
    hbias = np.zeros((34, 1), np.float32)
    hbias[0, 0] = AC * b4[0] + A0
    hbias[1, 0] = BC * b4[1] + B0
    identm = np.eye(128, dtype=np.float16)

    in_maps = []
    for c in range(N_CORES):
        cs = slice(c * NPC, (c + 1) * NPC)
        im = {
            "z1": np.ascontiguousarray(z1[:, cs]),
            "s1": np.ascontiguousarray(s1[:, cs]),
            "g1c": np.ascontiguousarray(g1c[:, cs]),
            "w2t": w2t, "w3t": w3t,
            "whz": whz, "whv": whv,
            "bias": biasm, "hbias": hbias, "ident": identm,
        }
        for d in range(3):
            im[f"wct{d}"] = wct[d]
        in_maps.append(im)
    return in_maps


_NC_CACHE = None


def get_program():
    global _NC_CACHE
    if _NC_CACHE is None:
        _NC_CACHE = _build_program()
    return _NC_CACHE


def kernel(**inputs) -> np.ndarray:
    nc = get_program()
    in_maps = _host_prep(inputs)
    r = run_bass_kernel_spmd(nc, in_maps, core_ids=list(range(N_CORES)))
    out4 = np.concatenate(
        [r.results[c]["out"] for c in range(N_CORES)], axis=1
    ).astype(np.float32)
    f = np.asarray(inputs["f"], np.float64)
    kf = ((_GS * f + _GB) ** 2).astype(np.float32)
    return out4[2:4] + kf[None, :] * out4[0:2]


# revision 21
# speedup vs baseline: 1.2024x; 1.0100x over previous
"""Trainium2 Bass kernel for the BVPModel Helmholtz-residual PINN problem.

Computes, for N=131072 collocation points, the residual of a Helmholtz-type
PDE through a 4-256-256-256-2 tanh MLP, including the beta-weighted sum of
diagonal second derivatives w.r.t. the three spatial inputs.

Math: forward-mode second-order jets (forward Laplacian). Per point:
  - primal       h_l
  - tangents     u_l^d (scaled by sqrt(beta_d), d=x,y,z)
  - second-order v_l   (beta-weighted; sign/scale folded into head)
through the layers:
  affine:  hat = W h,   uhat_d = W u_d,   vhat = W v
  tanh:    z = tanh(hat+b), s = 1 - z^2
           u_d = s * uhat_d
           v   = (z * sum_d uhat_d^2 + vhat) * s
Folds (all host-side, free on device):
  - a1 = W1 @ [x,y,z,f] + b1 computed on host, streamed in as fp16
  - uhat2_d = (W2 diag(chat_d)) @ s1  -> three Wc_d weight matrices
  - vhat2   = (W2 diag(cc)) @ (z1*s1) -> streamed g1c with plain W2
  - kf = (gs*f+gb)^2 squared on host; AC/BC folded into head weights
Head:
  m1 = (diag(AC,BC) W4) z3 + hb4 ;  c2 = (diag(AC,BC)(-2 W4)) v3
  host: res = c2 + kf*m1

Engine balance (the point of this version): the baseline saturated the
scalar engine (~25us/tile of ACTIVATEs) while GpSimd idled.  Here every
PSUM tile has exactly ONE reader, the elementwise jet chain is spread
across ACT / DVE / GpSimd (~17us/tile each), fused custom DVE ops
(SQSQ/ADDSQ) halve the sum-of-squares chains, and z3*q3 is accumulated
into the pv3 PSUM group via an identity matmul so the layer-3 v-channel
needs no separate PSUM-add pass.  Layer-3 tail (pv3 + head) of tile j is
emitted inside tile j+1's instruction stream so the PE never waits on the
elementwise chain.

Sharding: pure data parallel over 8 NeuronCores (16384 points each),
weights replicated, no collectives.
"""

import math
from contextlib import ExitStack

import numpy as np

import concourse.bass as bass
import concourse.bacc as bacc
import concourse.mybir as mybir
import concourse.tile as tile
import concourse.bass_utils as bass_utils
from concourse.bass_utils import run_bass_kernel_spmd

from concourse import dve_ops as _DO
from concourse.dve_spec import Spec as _Spec, Src0 as _Src0, Src1 as _Src1, \
    One as _One, lower as _lower
from concourse.dve_uop import DveOpSpec as _DveOpSpec


def _register_op(name, spec, subdim=False):
    if name in _DO._SUB_OPCODE_FOR_NAME:
        for op in _DO.OPS:
            if op.name == name:
                return op
    shas = {}
    opcode = max(_DO._SUB_OPCODE_FOR_NAME.values()) + 1
    for ver in ("v3", "v4"):
        try:
            uops = _lower(spec, ver=ver)
        except Exception:
            continue
        tmp = _DveOpSpec(name=name, opcode=opcode, uops=uops,
                         rd1_en=_DO.has_src1(spec))
        shas[ver] = tmp.sha(ver)
    op = _DO.DveOp(name, spec, subdim, shas)
    _DO.OPS.append(op)
    _DO._SUB_OPCODE_FOR_NAME[name] = opcode
    return op


# v = (1 - z^2) * w   (Src0 = z, Src1 = w)
BVP_SMUL = _register_op(
    "BVP_SMUL_ANT",
    _Spec(body=(_One - _Src0 * _Src0) * _Src1,
          reference=lambda in0, in1, c0, c1, c2: (1.0 - in0 * in0) * in1),
)

# q = a^2 + b^2
BVP_SQSQ = _register_op(
    "BVP_SQSQ_ANT",
    _Spec(body=_Src0 * _Src0 + _Src1 * _Src1,
          reference=lambda in0, in1, c0, c1, c2: in0 * in0 + in1 * in1),
)

# q = a + b^2
BVP_ADDSQ = _register_op(
    "BVP_ADDSQ_ANT",
    _Spec(body=_Src0 + _Src1 * _Src1,
          reference=lambda in0, in1, c0, c1, c2: in0 + in1 * in1),
)

# ---- problem constants (from the BVPModel definition) ----
_C0 = 343.0
FC, F0 = 500.0, 100.0
XC, YC, ZC = 0.7, 0.5, 0.6
AC, A0 = 2.0, 0.1
BC, B0 = 1.5, -0.05
BETA = ((YC * ZC) ** 2, (XC * ZC) ** 2, (XC * YC) ** 2)

_GS = np.float32(2.0 * math.pi * FC * (XC * YC * ZC) / _C0)
_GB = np.float32(2.0 * math.pi * F0 * (XC * YC * ZC) / _C0)

N_TOTAL = 131072
N_CORES = 8
NPC = N_TOTAL // N_CORES  # 16384 points per core
H = 256
B = 1024                  # points per tile
NT = NPC // B             # tiles per core

F32 = mybir.dt.float32
F16 = mybir.dt.float16

Alu = mybir.AluOpType
Act = mybir.ActivationFunctionType


def _build_program():
    """Build the per-core Bass program (identical on all 8 cores)."""
    nc = bacc.Bacc("TRN2", target_bir_lowering=False, debug=False)

    # ---- DRAM I/O ----
    d_z1 = nc.dram_tensor("z1", [H, NPC], F16, kind="ExternalInput").ap()
    d_s1 = nc.dram_tensor("s1", [H, NPC], F16, kind="ExternalInput").ap()
    d_g1c = nc.dram_tensor("g1c", [H, NPC], F16, kind="ExternalInput").ap()
    d_w2t = nc.dram_tensor("w2t", [H, H], F16, kind="ExternalInput").ap()
    d_w3t = nc.dram_tensor("w3t", [H, H], F16, kind="ExternalInput").ap()
    d_wct = [
        nc.dram_tensor(f"wct{d}", [H, H], F16, kind="ExternalInput").ap()
        for d in range(3)
    ]
    d_whz = nc.dram_tensor("whz", [H, 2], F16, kind="ExternalInput").ap()
    d_whv = nc.dram_tensor("whv", [H, 2], F16, kind="ExternalInput").ap()
    d_bias = nc.dram_tensor("bias", [H, 2], F32, kind="ExternalInput").ap()
    d_hbias = nc.dram_tensor("hbias", [34, 1], F32, kind="ExternalInput").ap()
    d_ident = nc.dram_tensor("ident", [128, 128], F16,
                             kind="ExternalInput").ap()
    d_out = nc.dram_tensor("out", [4, NPC], F16, kind="ExternalOutput").ap()

    with tile.TileContext(nc) as tc, ExitStack() as ctx:
        singles = ctx.enter_context(tc.tile_pool(name="singles", bufs=1))
        work = ctx.enter_context(tc.tile_pool(name="work", bufs=1))
        psum = ctx.enter_context(tc.tile_pool(name="psum", bufs=1,
                                              space="PSUM"))

        # ---- load weights / constants once ----
        w2t = [singles.tile([128, H], F16, name=f"w2t{k}") for k in range(2)]
        w3t = [singles.tile([128, H], F16, name=f"w3t{k}") for k in range(2)]
        wct = [[singles.tile([128, H], F16, name=f"wct{d}_{k}")
                for k in range(2)] for d in range(3)]
        whz = [singles.tile([128, 2], F16, name=f"whz{k}") for k in range(2)]
        whv = [singles.tile([128, 2], F16, name=f"whv{k}") for k in range(2)]
        bias = [singles.tile([128, 2], F32, name=f"bias{k}") for k in range(2)]
        for k in range(2):
            sl = slice(k * 128, (k + 1) * 128)
            nc.sync.dma_start(out=w2t[k], in_=d_w2t[sl, :])
            nc.sync.dma_start(out=w3t[k], in_=d_w3t[sl, :])
            for d in range(3):
                nc.sync.dma_start(out=wct[d][k], in_=d_wct[d][sl, :])
            nc.sync.dma_start(out=whz[k], in_=d_whz[sl, :])
            nc.sync.dma_start(out=whv[k], in_=d_whv[sl, :])
            nc.sync.dma_start(out=bias[k], in_=d_bias[sl, :])
        hbias = singles.tile([34, 1], F32)
        nc.sync.dma_start(out=hbias, in_=d_hbias)
        ident = singles.tile([128, 128], F16)
        nc.sync.dma_start(out=ident, in_=d_ident)

        HS = [slice(0, 512), slice(512, 1024)]

        def emit_finish(p):
            """Layer-3 v-channel tail + head for a previous tile.

            pv3 = W3 @ v2 (+ I @ t3, accumulated on the PE) per 512-chunk,
            then v3 = (1 - z3^2) * pv3 and the packed 2-col-group head.
            """
            j, z3, v2, t3 = p["j"], p["z3"], p["v2"], p["t3"]
            js = slice(j * B, (j + 1) * B)
            v3 = [work.tile([128, B], F16, name=f"v3_{m}", bufs=2)
                  for m in range(2)]
            for m in range(2):
                msl = slice(m * 128, (m + 1) * 128)
                pz = psum.tile([128, B], F32, tag="pmain", bufs=3,
                               name=f"pv3_{m}")
                for c in range(2):
                    hs = HS[c]
                    nc.tensor.matmul(pz[:, hs], w3t[0][:, msl], v2[0][:, hs],
                                     start=True, stop=False)
                    nc.tensor.matmul(pz[:, hs], w3t[1][:, msl], v2[1][:, hs],
                                     start=False, stop=False)
                    nc.tensor.matmul(pz[:, hs], ident, t3[m][:, hs],
                                     start=False, stop=True)
                nc.vector._custom_dve(BVP_SMUL, out=v3[m], in0=z3[m], in1=pz)
            hout = work.tile([34, B], F16, name="hout", bufs=2)
            ph34 = psum.tile([34, B], F32, tag="phead", bufs=1, name="ph34")
            for c in range(2):
                hs = HS[c]
                for k in range(2):
                    nc.tensor.matmul(ph34[0:2, hs], whz[k], z3[k][:, hs],
                                     start=(k == 0), stop=(k == 1))
                for k in range(2):
                    nc.tensor.matmul(ph34[32:34, hs], whv[k], v3[k][:, hs],
                                     start=(k == 0), stop=(k == 1))
            nc.scalar.activation(hout, ph34, Act.Identity, bias=hbias)
            nc.sync.dma_start(out=d_out[0:2, js], in_=hout[0:2, :])
            nc.sync.dma_start(out=d_out[2:4, js], in_=hout[32:34, :])

        prev = None
        for j in range(NT):
            js = slice(j * B, (j + 1) * B)

            # ---------- stream layer-1 jet (host-computed) ----------
            z1, s1, g1 = [], [], []
            for k in range(2):
                sl = slice(k * 128, (k + 1) * 128)
                z = work.tile([128, B], F16, name=f"z1_{k}", bufs=2)
                nc.sync.dma_start(out=z, in_=d_z1[sl, js])
                s = work.tile([128, B], F16, name=f"s1_{k}", bufs=2)
                nc.sync.dma_start(out=s, in_=d_s1[sl, js])
                g = work.tile([128, B], F16, name=f"g1_{k}", bufs=2)
                nc.sync.dma_start(out=g, in_=d_g1c[sl, js])
                z1.append(z)
                s1.append(s)
                g1.append(g)

            # ---------- layer 2, pass 1: h-channel both halves ----------
            z2, s2 = [], []
            for m in range(2):
                msl = slice(m * 128, (m + 1) * 128)
                ph = psum.tile([128, B], F32, tag="pmain", bufs=3,
                               name=f"ph2_{m}")
                for k in range(2):
                    for c in range(2):
                        nc.tensor.matmul(ph[:, HS[c]], w2t[k][:, msl],
                                         z1[k][:, HS[c]],
                                         start=(k == 0), stop=(k == 1))
                z = work.tile([128, B], F16, name=f"z2_{m}")
                nc.scalar.activation(z, ph, Act.Tanh, bias=bias[m][:, 0:1])
                zq = work.tile([128, B], F16, name=f"zq2_{m}")
                nc.vector.tensor_mul(zq, z, z)
                s = work.tile([128, B], F16, name=f"s2_{m}")
                nc.vector.tensor_scalar(s, zq, -1.0, 1.0, Alu.mult, Alu.add)
                z2.append(z)
                s2.append(s)

            # ---------- layer 2, pass 2: tangent channels ----------
            # u-muls for BOTH halves precede the q-chains in the DVE FIFO:
            # u2 gates layer-3 MMs, while the q-chain output (tl2) is only
            # consumed by the deferred pv2/STT block below.
            u2, tl2l, uhs = [[None] * 2 for _ in range(3)], [], []
            for m in range(2):
                msl = slice(m * 128, (m + 1) * 128)
                uh = []
                for d in range(3):
                    pu = psum.tile([128, B], F32, tag="pmain", bufs=3,
                                   name=f"pu2_{d}{m}")
                    for k in range(2):
                        for c in range(2):
                            nc.tensor.matmul(pu[:, HS[c]], wct[d][k][:, msl],
                                             s1[k][:, HS[c]],
                                             start=(k == 0), stop=(k == 1))
                    t = work.tile([128, B], F16, name=f"uh2_{d}{m}")
                    nc.scalar.activation(t, pu, Act.Copy)
                    uh.append(t)
                for d in range(3):
                    u = work.tile([128, B], F16, name=f"u2_{d}{m}")
                    nc.vector.tensor_mul(u, uh[d], s2[m])
                    u2[d][m] = u
                uhs.append(uh)
            for m in range(2):
                uh = uhs[m]
                q1 = work.tile([128, B], F16, name=f"q1l2_{m}")
                nc.vector._custom_dve(BVP_SQSQ, out=q1, in0=uh[0], in1=uh[1])
                q2 = work.tile([128, B], F16, name=f"q2l2_{m}")
                nc.vector._custom_dve(BVP_ADDSQ, out=q2, in0=q1, in1=uh[2])
                tl2 = work.tile([128, B], F16, name=f"tl2_{m}")
                nc.vector.tensor_mul(tl2, q2, z2[m])
                tl2l.append(tl2)

            # ---------- layer 3 (h + tangents; v-tail deferred) ----------
            z3, t3 = [], []
            for m in range(2):
                msl = slice(m * 128, (m + 1) * 128)
                ph = psum.tile([128, B], F32, tag="pmain", bufs=3,
                               name=f"ph3_{m}")
                pu = [psum.tile([128, B], F32, tag="pmain", bufs=3,
                                name=f"pu3_{d}{m}") for d in range(3)]
                for k in range(2):
                    for c in range(2):
                        nc.tensor.matmul(ph[:, HS[c]], w3t[k][:, msl],
                                         z2[k][:, HS[c]],
                                         start=(k == 0), stop=(k == 1))
                    for d in range(3):
                        for c in range(2):
                            nc.tensor.matmul(pu[d][:, HS[c]],
                                             w3t[k][:, msl],
                                             u2[d][k][:, HS[c]],
                                             start=(k == 0), stop=(k == 1))
                z = work.tile([128, B], F16, name=f"z3_{m}", bufs=2)
                nc.scalar.activation(z, ph, Act.Tanh, bias=bias[m][:, 1:2])
                ysx = work.tile([128, B], F16, name=f"ysx_{m}")
                nc.scalar.activation(ysx, pu[0], Act.Square)
                ysy = work.tile([128, B], F16, name=f"ysy_{m}")
                nc.scalar.activation(ysy, pu[1], Act.Square)
                ysz = work.tile([128, B], F16, name=f"ysz_{m}")
                nc.scalar.activation(ysz, pu[2], Act.Square)
                q13 = work.tile([128, B], F16, name=f"q13_{m}")
                nc.vector.tensor_add(q13, ysx, ysy)
                q3 = work.tile([128, B], F16, name=f"q3_{m}")
                nc.vector.tensor_add(q3, q13, ysz)
                t = work.tile([128, B], F16, name=f"t3_{m}", bufs=2)
                nc.gpsimd.tensor_mul(t, q3, z)
                z3.append(z)
                t3.append(t)

            # ---------- layer-3 tail + head of the previous tile ----------
            if prev is not None:
                emit_finish(prev)

            # ---------- layer-2 v-channel (input-only; emitted LAST so its
            # PSUM slots — held until the DVE STT fires — recycle into the
            # next tile's ph2 allocations rather than blocking pv3/head.
            # v2 itself is not needed until the NEXT tile's finish block) ---
            v2 = []
            for m in range(2):
                msl = slice(m * 128, (m + 1) * 128)
                pv = psum.tile([128, B], F32, tag="pmain", bufs=3,
                               name=f"pv2_{m}")
                for k in range(2):
                    for c in range(2):
                        nc.tensor.matmul(pv[:, HS[c]], w2t[k][:, msl],
                                         g1[k][:, HS[c]],
                                         start=(k == 0), stop=(k == 1))
                t2 = work.tile([128, B], F16, name=f"t2l2_{m}")
                nc.vector.scalar_tensor_tensor(t2, tl2l[m], 1.0, pv,
                                               op0=Alu.mult, op1=Alu.add)
                v = work.tile([128, B], F16, name=f"v2_{m}", bufs=2)
                nc.gpsimd.tensor_mul(v, t2, s2[m])
                v2.append(v)

            prev = {"j": j, "z3": z3, "v2": v2, "t3": t3}

        emit_finish(prev)

    nc.compile()
    return nc


def _host_prep(inputs):
    """Host-side: layer-1 forward jet + weight folds + sharding."""
    x = np.asarray(inputs["x"], np.float32)
    y = np.asarray(inputs["y"], np.float32)
    z = np.asarray(inputs["z"], np.float32)
    f = np.asarray(inputs["f"], np.float32)
    W1 = np.asarray(inputs["W1"], np.float32)
    b1 = np.asarray(inputs["b1"], np.float32)
    W2 = np.asarray(inputs["W2"], np.float32)
    b2 = np.asarray(inputs["b2"], np.float32)
    W3 = np.asarray(inputs["W3"], np.float32)
    b3 = np.asarray(inputs["b3"], np.float32)
    W4 = np.asarray(inputs["W4"], np.float32)
    b4 = np.asarray(inputs["b4"], np.float32)

    sb = np.sqrt(np.asarray(BETA, np.float64))
    chat = (sb[None, :] * W1[:, :3].astype(np.float64)).astype(np.float32)
    cc = (np.asarray(BETA)[None, :] * W1[:, :3].astype(np.float64) ** 2) \
        .sum(1).astype(np.float32)                              # [256]

    xyzf = np.stack([x, y, z, f])                               # [4, N] f32
    a1 = W1 @ xyzf + b1[:, None]
    z1f = np.tanh(a1)
    s1f = (1.0 - z1f * z1f).astype(np.float32)
    g1cf = (cc[:, None] * z1f * s1f).astype(np.float32)

    z1 = z1f.astype(np.float16)
    s1 = s1f.astype(np.float16)
    g1c = g1cf.astype(np.float16)

    biasm = np.ascontiguousarray(np.stack([b2, b3], axis=1))    # [256, 2]
    w2t = np.ascontiguousarray(W2.T.astype(np.float16))
    w3t = np.ascontiguousarray(W3.T.astype(np.float16))
    wct = [
        np.ascontiguousarray((W2 * chat[None, :, d]).T.astype(np.float16))
        for d in range(3)
    ]
    ab = np.array([AC, BC], np.float32)
    whz = np.ascontiguousarray((ab[:, None] * W4).T.astype(np.float16))
    whv = np.ascontiguousarray((ab[:, None] * (-2.0 * W4)).T
                               .astype(np.float16))
    hbias = np.zeros((34, 1), np.float32)
    hbias[0, 0] = AC * b4[0] + A0
    hbias[1, 0] = BC * b4[1] + B0
    identm = np.eye(128, dtype=np.float16)

    in_maps = []
    for c in range(N_CORES):
        cs = slice(c * NPC, (c + 1) * NPC)
        im = {
            "z1": np.ascontiguousarray(z1[:, cs]),
            "s1": np.ascontiguousarray(s1[:, cs]),
            "g1c": np.ascontiguousarray(g1c[:, cs]),
            "w2t": w2t, "w3t": w3t,
            "whz": whz, "whv": whv,
            "bias": biasm, "hbias": hbias, "ident": identm,
        }
        for d in range(3):
            im[f"wct{d}"] = wct[d]
        in_maps.append(im)
    return in_maps


_NC_CACHE = None


def get_program():
    global _NC_CACHE
    if _NC_CACHE is None:
        _NC_CACHE = _build_program()
    return _NC_CACHE


def kernel(**inputs) -> np.ndarray:
    nc = get_program()
    in_maps = _host_prep(inputs)
    r = run_bass_kernel_spmd(nc, in_maps, core_ids=list(range(N_CORES)))
    out4 = np.concatenate(
        [r.results[c]["out"] for c in range(N_CORES)], axis=1
    ).astype(np.float32)
    f = np.asarray(inputs["f"], np.float64)
    kf = ((_GS * f + _GB) ** 2).astype(np.float32)
    return out4[2:4] + kf[None, :] * out4[0:2]
